# revision 13
# baseline (speedup 1.0000x reference)
"""Multi-head attention (B=2, S=2048, H=1024, NH=16, DK=DV=64) on 8 TRN2 cores.

Sharding: data-parallel over batch (2 groups of 4 cores) x tensor-parallel
over heads (4 heads per core, as 2 pairs of 2).  Each core computes, for its
batch sample and its 4 heads:

    Q^T/K^T projections (features on partitions), V projection (natural),
    S^T = K @ Q^T per 128-key chunk (causal chunks only, 2 heads fused into
    one 2-bank PSUM tile),
    P^T = exp(S^T/8 + pad_bias)   (one fused exp over both heads; no
    max-subtraction needed: |scores| ~ N(0,1)),
    out^T = V_aug^T @ P^T  where V_aug = [V | ones] for head A and
    [ones | V] for head B, so oA = [attnA; denA] and oB = [denB; attnB].
    A single PE matmul with a half-swap permutation aligns both
    denominators with their numerators; one reciprocal + two muls
    normalize directly into attnT.
    y_partial = attnT^T @ W_O_rows   (row-sharded W_O).

Host sums the 4 partials per batch and adds (b_V @ W_O + b_O) (exact since
softmax rows sum to 1).

Pipelining: x^T is DMA'd in query-column blocks (split into <=256KB pieces
so no single DMA queue gates the PE start); projections for query block
jb+1 and the output projection for block j-1 are emitted interleaved into
block j's attention chunk stream so the PE never idles while the
activation engine works through the exps.  PSUM: 4 banks score/proj
rotation + 2 banks out-proj/denominator rotation + 2 banks attnV
accumulators = 8.
"""

import math
from contextlib import ExitStack

import ml_dtypes
import numpy as np


import concourse.bass as bass
import concourse.mybir as mybir
from concourse import bacc
import concourse.tile as tile
from concourse.bass_utils import run_bass_kernel_spmd

F32 = mybir.dt.float32
F32R = mybir.dt.float32r
BF16 = mybir.dt.bfloat16
EXP = mybir.ActivationFunctionType.Exp

B, S, H = 2, 2048, 1024
NH, DK, DV = 16, 64, 64
NCORE = 8
NCH = H // 128          # 8 contraction chunks over H
NJ = S // 512           # 4 query blocks of 512
NKC = S // 128          # 16 key chunks
NPAIR = 2               # head pairs per core
SCALE = 1.0 / math.sqrt(DK)
NEG_BIAS = -30000.0     # exp(x + NEG_BIAS) == 0.0 in fp32 for any real score


def _r(ap):
    """Bitcast an fp32 AP to float32r so the PE runs at 1 cycle/row."""
    return ap.bitcast(F32R)


def _emit(nc, d):
    """Emit the per-core program.  d maps names -> DRAM tensor handles."""
    with tile.TileContext(nc) as tc, ExitStack() as top:
        consts = top.enter_context(tc.tile_pool(name="consts", bufs=1))
        persist = top.enter_context(tc.tile_pool(name="persist", bufs=1))

        # ---- tiles for constants / weights ----
        bq_sb = consts.tile([128, 2], F32, tag="bq", name="bqsb")
        bk_sb = consts.tile([128, 2], F32, tag="bk", name="bksb")
        nbias_sb = consts.tile([128, NKC], F32, tag="nbias", name="nbiassb")
        mdiag_f32 = consts.tile([128, 2, 128], F32, tag="mdf", name="mdiagf32")
        mdiag_sb = consts.tile([128, 2, 128], BF16, tag="mdiag", name="mdiagsb")
        swap_sb = consts.tile([128, 128], F32R, tag="swap", name="swapsb")
        wqq_sb = [consts.tile([128, NCH * 128], BF16, tag=f"wqq{p}",
                              name=f"wqq{p}sb") for p in range(NPAIR)]
        wkk_sb = [consts.tile([128, NCH * 128], BF16, tag=f"wkk{p}",
                              name=f"wkk{p}sb") for p in range(NPAIR)]
        wv_sb = consts.tile([128, NCH * 256], BF16, tag="wv", name="wvsb")
        wo_sb = consts.tile([128, 2 * 1024], F32R, tag="wo", name="wosb")
        xt_sb = [persist.tile([128, S], BF16, tag=f"xt{c}", name=f"xt{c}sb")
                 for c in range(NCH)]

        def dma_cols(sb, dram, lo, hi, pieces, bitcast=False, eng=None):
            eng = eng if eng is not None else nc.sync
            step_c = (hi - lo) // pieces
            for i in range(pieces):
                a, b_ = lo + i * step_c, lo + (i + 1) * step_c
                src = dram[:, a:b_]
                eng.dma_start(out=sb[:, a:b_],
                              in_=src.bitcast(F32R) if bitcast else src)

        # ---- input DMAs: weights on the SP DGE, x^T on the Activation DGE
        # (two hardware descriptor generators issue in parallel; ~0.6us per
        # dma_start on each sequencer is what paces the prologue) ----
        xtd = [d["xt"][c * 128:(c + 1) * 128, :] for c in range(NCH)]
        dma_cols(wqq_sb[0], d["wqq"][0], 0, 256, 2)
        dma_cols(xt_sb[0], xtd[0], 0, 512, 2, eng=nc.scalar)
        dma_cols(wqq_sb[0], d["wqq"][0], 256, 1024, 3)
        for c in range(1, NCH):
            dma_cols(xt_sb[c], xtd[c], 0, 512, 1, eng=nc.scalar)
        nc.sync.dma_start(out=bq_sb, in_=d["bq"][:])
        nc.sync.dma_start(out=bk_sb, in_=d["bk"][:])
        nc.sync.dma_start(out=nbias_sb, in_=d["nbias"][:])
        nc.sync.dma_start(out=mdiag_f32, in_=d["mdiag2"][:])
        nc.vector.tensor_copy(mdiag_sb, mdiag_f32)
        nc.sync.dma_start(out=swap_sb, in_=d["swap"][:].bitcast(F32R))
        dma_cols(wkk_sb[0], d["wkk"][0], 0, 1024, 4)
        dma_cols(wv_sb, d["wv"], 0, 2048, 4, eng=nc.scalar)
        dma_cols(wqq_sb[1], d["wqq"][1], 0, 1024, 4)
        dma_cols(wkk_sb[1], d["wkk"][1], 0, 1024, 4)
        for c in range(NCH):
            dma_cols(xt_sb[c], xtd[c], 512, 1024, 1)
        dma_cols(wo_sb, d["wo"], 0, 2048, 4, bitcast=True)
        for jb in (2, 3):
            for c in range(NCH):
                dma_cols(xt_sb[c], xtd[c], jb * 512, (jb + 1) * 512, 1)

        # ---- persistent activations ----
        qt_sb = []   # per pair: [128, S]; rows 0:64 head A Q^T, 64:128 head B
        kt_sb = []
        attnT = []   # per pair: [128, S]; normalized attn^T (dims on rows)
        for p in range(NPAIR):
            qt_sb.append(persist.tile([128, S], BF16, tag=f"qt{p}",
                                      name=f"qt{p}sb"))
            kt_sb.append(persist.tile([128, S], BF16, tag=f"kt{p}",
                                      name=f"kt{p}sb"))
            attnT.append(persist.tile([128, S], F32R, tag=f"at{p}",
                                      name=f"at{p}sb"))
        # V_aug [128, parity, pair, keycols]: even heads (A) = [V | ones],
        # odd heads (B) = [ones | V]
        vaug = persist.tile([128, 2, 2, NKC * 128], BF16, tag="vaug",
                            name="vaugsb")
        nc.gpsimd.memset(vaug, 1.0)

        # ---- PSUM pools: 4 (scores+proj) + 2 (outproj/den) + 2 (oA,oB) ----
        sp = top.enter_context(tc.tile_pool(name="sp", bufs=2, space="PSUM"))
        rot = top.enter_context(tc.tile_pool(name="rot", bufs=2, space="PSUM"))
        op = top.enter_context(tc.tile_pool(name="op", bufs=1, space="PSUM"))

        # ---- SBUF work pools ----
        ptp = top.enter_context(tc.tile_pool(name="ptp", bufs=6))
        nrm = top.enter_context(tc.tile_pool(name="nrm", bufs=3))
        ysb = top.enter_context(tc.tile_pool(name="ysb", bufs=4))

        def proj_units(jb):
            """Q/K/V projection for query block jb: 8 independent units."""
            jsl = slice(jb * 512, (jb + 1) * 512)
            units = []
            for p in range(NPAIR):
                for wsb, bsb, dst, nm in (
                    (wqq_sb[p], bq_sb, qt_sb[p], "q"),
                    (wkk_sb[p], bk_sb, kt_sb[p], "k"),
                ):
                    def u(p=p, wsb=wsb, bsb=bsb, dst=dst, nm=nm, jsl=jsl, jb=jb):
                        ps = sp.tile([128, 2, 512], F32, tag="s",
                                     name=f"ps{nm}{p}{jb}")
                        for c in range(NCH):
                            nc.tensor.matmul(
                                ps[:, 0, :],
                                wsb[:, c * 128:(c + 1) * 128],
                                xt_sb[c][:, jsl],
                                start=(c == 0), stop=(c == NCH - 1),
                            )
                        nc.vector.tensor_scalar_add(dst[:, jsl], ps[:, 0, :],
                                                    bsb[:, p:p + 1])
                    units.append(u)
            for t in range(4 * jb, 4 * jb + 4):
                def u(t=t):
                    ps = sp.tile([128, 4, 128], F32, tag="s", name=f"psv{t}")
                    for c in range(NCH):
                        nc.tensor.matmul(
                            ps[:, 0:2, :],
                            xt_sb[c][:, t * 128:(t + 1) * 128],
                            wv_sb[:, c * 256:(c + 1) * 256],
                            start=(c == 0), stop=(c == NCH - 1),
                        )
                    # ps cols = [h0|h1|h2|h3] x 64; even heads' V to parity 0
                    # front half, odd heads' V to parity 1 back half.
                    nc.vector.tensor_copy(
                        vaug[:, 0, :, t * 128:t * 128 + 64], ps[:, 0:2, 0:64])
                    nc.vector.tensor_copy(
                        vaug[:, 1, :, t * 128 + 64:(t + 1) * 128],
                        ps[:, 0:2, 64:128])
                units.append(u)
            return units

        def psf_units(j, tail=False):
            """Output projection for query block j: 8 independent units."""
            units = []
            for q in range(4 * j, 4 * j + 4):
                for half in range(2):
                    def u(q=q, half=half, tail=tail):
                        pf = rot.tile([128, 512], F32, tag="r",
                                      name=f"pf{q}{half}")
                        for p in range(NPAIR):
                            nc.tensor.matmul(
                                pf,
                                _r(attnT[p][:, q * 128:(q + 1) * 128]),
                                _r(wo_sb[:, p * 1024 + half * 512:
                                         p * 1024 + half * 512 + 512]),
                                start=(p == 0), stop=(p == 1),
                            )
                        yt = ysb.tile([128, 512], BF16, tag="y",
                                      name=f"yt{q}{half}")
                        if tail:
                            nc.scalar.copy(yt, pf)
                            eng = nc.sync if (q + half) % 2 else nc.scalar
                            eng.dma_start(
                                out=d["y"][q * 128:(q + 1) * 128,
                                           half * 512:(half + 1) * 512],
                                in_=yt)
                        else:
                            nc.vector.tensor_copy(yt, pf)
                            for piece in range(2):
                                ysl = slice(piece * 256, (piece + 1) * 256)
                                nc.gpsimd.dma_start(
                                    out=d["y"][q * 128:(q + 1) * 128,
                                               half * 512 + piece * 256:
                                               half * 512 + (piece + 1) * 256],
                                    in_=yt[:, ysl])
                    units.append(u)
            return units

        def emit_scores(p, j, c):
            """Scores + exp (+ diag mask) for chunk c; returns attnV args."""
            t = c - 4 * j
            fo = 128 * t if t > 0 else 0
            w = 512 - fo
            qsl = slice(j * 512 + fo, (j + 1) * 512)
            s2 = sp.tile([128, 2, 512], F32, tag="s", name=f"s{p}{j}{c}")
            nc.tensor.matmul(s2[:, 0:1, :w],
                             kt_sb[p][0:64, c * 128:(c + 1) * 128],
                             qt_sb[p][0:64, qsl], start=True, stop=True)
            nc.tensor.matmul(s2[:, 1:2, :w],
                             kt_sb[p][64:128, c * 128:(c + 1) * 128],
                             qt_sb[p][64:128, qsl], start=True, stop=True)
            p2 = ptp.tile([128, 2, 512], BF16, tag="p", name=f"p{p}{j}{c}")
            nc.scalar.activation(p2[:, :, :w], s2[:, :, :w], EXP,
                                 bias=nbias_sb[:, c:c + 1], scale=SCALE)
            if t >= 0:
                # diagonal 128x128 block: zero keys below the diagonal for
                # both heads in one op
                nc.vector.tensor_mul(p2[:, :, 0:128], p2[:, :, 0:128],
                                     mdiag_sb)
            return p2, fo, w

        def emit_attnv(p, j, c, oA, oB, cmax, p2, fo, w):
            ksl = slice(c * 128, (c + 1) * 128)
            nc.tensor.matmul(oA[:, fo:512], vaug[:, 0, p, ksl],
                             p2[:, 0:1, :w], start=(c == 0), stop=(c == cmax))
            nc.tensor.matmul(oB[:, fo:512], vaug[:, 1, p, ksl],
                             p2[:, 1:2, :w], start=(c == 0), stop=(c == cmax))

        def emit_norm(p, j, oA, oB):
            # denA = oA[64:128], denB = oB[0:64]; swap halves on the PE so
            # each reciprocal lands on its numerator's partitions.
            jsl = slice(j * 512, (j + 1) * 512)
            scr = nrm.tile([128, 512], F32R, tag="scr", name=f"scr{p}{j}")
            nc.vector.tensor_copy(scr[64:128, :], oA[64:128, :])
            nc.vector.tensor_copy(scr[0:64, :], oB[0:64, :])
            den2 = rot.tile([128, 512], F32, tag="r", name=f"den{p}{j}")
            nc.tensor.matmul(den2, swap_sb, scr, start=True, stop=True)
            rec = nrm.tile([128, 512], F32, tag="rec", name=f"rec{p}{j}")
            nc.vector.reciprocal_approx_fast(out=rec, in_=den2)
            nc.vector.tensor_mul(attnT[p][0:64, jsl], oA[0:64, :],
                                 rec[0:64, :])
            nc.vector.tensor_mul(attnT[p][64:128, jsl], oB[64:128, :],
                                 rec[64:128, :])

        # ---- main schedule ----
        for u in proj_units(0):
            u()
        # Filler assignment keeps every step PE-bound: proj(j+1) during
        # step j, and the out-projections pushed two steps later (j=3 has
        # the most exp work, so it gets psf(1)+psf(2) as extra PE filler).
        for j in range(NJ):
            fillers = []
            if j + 1 < NJ:
                fillers += proj_units(j + 1)
            if j == 1:
                fillers += psf_units(0)
            elif j == NJ - 1:
                fillers += psf_units(1) + psf_units(2)
            nch_j = 4 * j + 4
            total_chunks = 2 * nch_j
            done = 0
            ci = 0
            for p in range(NPAIR):
                oA = op.tile([128, 512], F32, tag="oA", name=f"oA{p}{j}")
                oB = op.tile([128, 512], F32, tag="oB", name=f"oB{p}{j}")
                pend = []
                for c in range(nch_j):
                    pend.append((c,) + emit_scores(p, j, c))
                    if len(pend) > 2:
                        c0, p2, fo, w = pend.pop(0)
                        emit_attnv(p, j, c0, oA, oB, nch_j - 1, p2, fo, w)
                    ci += 1
                    want = ci * len(fillers) // total_chunks
                    while done < want:
                        fillers[done]()
                        done += 1
                for c0, p2, fo, w in pend:
                    emit_attnv(p, j, c0, oA, oB, nch_j - 1, p2, fo, w)
                emit_norm(p, j, oA, oB)
            while done < len(fillers):
                fillers[done]()
                done += 1
        for u in psf_units(NJ - 1, tail=True):
            u()

        if _DEBUG:
            for p in range(NPAIR):
                nc.sync.dma_start(out=d[f"dbg_qt{p}"][:], in_=qt_sb[p].bitcast(F32))
                nc.sync.dma_start(out=d[f"dbg_kt{p}"][:], in_=kt_sb[p].bitcast(F32))
                nc.sync.dma_start(out=d[f"dbg_at{p}"][:], in_=attnT[p].bitcast(F32))
            for h in range(4):
                nc.sync.dma_start(out=d[f"dbg_va{h}"][:], in_=vaug[h].bitcast(F32))


_NC_CACHE = {}
_DEBUG = False


def _get_nc():
    if "nc" not in _NC_CACHE:
        nc = bacc.Bacc(None, target_bir_lowering=False)
        d = {
            "xt": nc.dram_tensor("xt", [H, S], BF16, kind="ExternalInput"),
            "wqq": nc.dram_tensor("wqq", [NPAIR, 128, NCH * 128], BF16,
                                  kind="ExternalInput"),
            "wkk": nc.dram_tensor("wkk", [NPAIR, 128, NCH * 128], BF16,
                                  kind="ExternalInput"),
            "wv": nc.dram_tensor("wv", [128, NCH * 256], BF16, kind="ExternalInput"),
            "wo": nc.dram_tensor("wo", [128, 2 * 1024], F32, kind="ExternalInput"),
            "bq": nc.dram_tensor("bq", [128, 2], F32, kind="ExternalInput"),
            "bk": nc.dram_tensor("bk", [128, 2], F32, kind="ExternalInput"),
            "nbias": nc.dram_tensor("nbias", [128, NKC], F32, kind="ExternalInput"),
            "mdiag2": nc.dram_tensor("mdiag2", [128, 2, 128], F32,
                                     kind="ExternalInput"),
            "swap": nc.dram_tensor("swap", [128, 128], F32, kind="ExternalInput"),
            "y": nc.dram_tensor("y", [S, H], BF16, kind="ExternalOutput"),
        }
        if _DEBUG:
            for p in range(NPAIR):
                d[f"dbg_qt{p}"] = nc.dram_tensor(f"dbg_qt{p}", [128, S], F32,
                                                 kind="ExternalOutput")
                d[f"dbg_kt{p}"] = nc.dram_tensor(f"dbg_kt{p}", [128, S], F32,
                                                 kind="ExternalOutput")
                d[f"dbg_at{p}"] = nc.dram_tensor(f"dbg_at{p}", [128, S], F32,
                                                 kind="ExternalOutput")
            for h in range(4):
                d[f"dbg_va{h}"] = nc.dram_tensor(f"dbg_va{h}", [128, NKC * 128],
                                                 F32, kind="ExternalOutput")
        _emit(nc, d)
        nc.finalize()
        _NC_CACHE["nc"] = nc
    return _NC_CACHE["nc"]


def _chunked(w, ncols):
    """[H, ncols] -> [128, NCH*ncols] with chunk c of rows at cols c*ncols."""
    return np.ascontiguousarray(
        w.reshape(NCH, 128, ncols).transpose(1, 0, 2).reshape(128, NCH * ncols))


def _make_in_maps(batch, input_ids, W_Q, W_K, W_V, W_O, b_Q, b_K):
    m = np.triu(np.ones((128, 128), np.float32))
    mdiag2 = np.ascontiguousarray(np.stack([m, m], axis=1))  # [128, 2, 128]
    swap = np.zeros((128, 128), np.float32)
    swap[64:128, 0:64] = np.eye(64, dtype=np.float32)
    swap[0:64, 64:128] = np.eye(64, dtype=np.float32)
    in_maps = []
    for core in range(NCORE):
        b, g = divmod(core, 4)
        base = 256 * g  # first feature column of this core's 4 heads
        wqq = np.stack([_chunked(W_Q[:, base + 128 * p: base + 128 * (p + 1)], 128)
                        for p in range(NPAIR)])
        wkk = np.stack([_chunked(W_K[:, base + 128 * p: base + 128 * (p + 1)], 128)
                        for p in range(NPAIR)])
        wv = _chunked(W_V[:, base: base + 256], 256)
        wo = np.ascontiguousarray(
            W_O[base: base + 256, :].reshape(2, 128, H)
            .transpose(1, 0, 2).reshape(128, 2 * H))
        bq = np.stack([b_Q[base + 128 * p: base + 128 * (p + 1)]
                       for p in range(NPAIR)], axis=1)
        bk = np.stack([b_K[base + 128 * p: base + 128 * (p + 1)]
                       for p in range(NPAIR)], axis=1)
        keep = input_ids[b] != 0
        nbias = np.where(keep, 0.0, NEG_BIAS).astype(np.float32)
        nbias = np.ascontiguousarray(nbias.reshape(NKC, 128).T)
        xt = np.ascontiguousarray(batch[b].T)
        bf = ml_dtypes.bfloat16
        in_maps.append({
            "xt": xt.astype(bf), "wqq": wqq.astype(bf),
            "wkk": wkk.astype(bf), "wv": wv.astype(bf), "wo": wo,
            "bq": np.ascontiguousarray(bq), "bk": np.ascontiguousarray(bk),
            "nbias": nbias, "mdiag2": mdiag2, "swap": swap,
        })
    return in_maps


def _run(in_maps, **kwargs):
    nc = _get_nc()
    return run_bass_kernel_spmd(nc, in_maps, core_ids=list(range(NCORE)), **kwargs)


def kernel(batch, input_ids, W_Q, W_K, W_V, b_Q, b_K, b_V, W_O, b_O,
           _results_out=None, **run_kwargs):
    batch = np.asarray(batch, np.float32)
    input_ids = np.asarray(input_ids)
    W_Q, W_K, W_V = (np.asarray(a, np.float32) for a in (W_Q, W_K, W_V))
    b_Q, b_K, b_V = (np.asarray(a, np.float32) for a in (b_Q, b_K, b_V))
    W_O = np.asarray(W_O, np.float32)
    b_O = np.asarray(b_O, np.float32)

    in_maps = _make_in_maps(batch, input_ids, W_Q, W_K, W_V, W_O, b_Q, b_K)
    res = _run(in_maps, **run_kwargs)
    if _results_out is not None:
        _results_out.append(res)
    ys = [np.asarray(res.results[c]["y"], np.float32) for c in range(NCORE)]
    out = np.stack([sum(ys[4 * b: 4 * b + 4]) for b in range(B)], axis=0)
    # b_V enters as attn@1 * b_V = b_V (softmax rows sum to 1), then @ W_O.
    const_row = (b_V @ W_O + b_O).astype(np.float32)
    return (out + const_row).astype(np.float32)


# revision 14
# speedup vs baseline: 1.0097x; 1.0097x over previous
"""Multi-head attention (B=2, S=2048, H=1024, NH=16, DK=DV=64) on 8 TRN2 cores.

Sharding: data-parallel over batch (2 groups of 4 cores) x tensor-parallel
over heads (4 heads per core, as 2 pairs of 2).  Each core computes, for its
batch sample and its 4 heads:

    Q^T/K^T projections (features on partitions), V projection (natural),
    S^T = K @ Q^T per 128-key chunk (causal chunks only, 2 heads fused into
    one 2-bank PSUM tile),
    P^T = exp(S^T/8 + pad_bias)   (one fused exp over both heads; no
    max-subtraction needed: |scores| ~ N(0,1)),
    out^T = V_aug^T @ P^T  where V_aug = [V | ones] for head A and
    [ones | V] for head B, so oA = [attnA; denA] and oB = [denB; attnB].
    A single PE matmul with a half-swap permutation aligns both
    denominators with their numerators; one reciprocal + two muls
    normalize directly into attnT.
    y_partial = attnT^T @ W_O_rows   (row-sharded W_O).

Host sums the 4 partials per batch and adds (b_V @ W_O + b_O) (exact since
softmax rows sum to 1).

Pipelining: x^T is DMA'd in query-column blocks (split into <=256KB pieces
so no single DMA queue gates the PE start); projections for query block
jb+1 and the output projection for block j-1 are emitted interleaved into
block j's attention chunk stream so the PE never idles while the
activation engine works through the exps.  PSUM: 4 banks score/proj
rotation + 2 banks out-proj/denominator rotation + 2 banks attnV
accumulators = 8.
"""

import math
from contextlib import ExitStack

import ml_dtypes
import numpy as np


import concourse.bass as bass
import concourse.mybir as mybir
from concourse import bacc
import concourse.tile as tile
from concourse.bass_utils import run_bass_kernel_spmd

F32 = mybir.dt.float32
F32R = mybir.dt.float32r
BF16 = mybir.dt.bfloat16
EXP = mybir.ActivationFunctionType.Exp

B, S, H = 2, 2048, 1024
NH, DK, DV = 16, 64, 64
NCORE = 8
NCH = H // 128          # 8 contraction chunks over H
NJ = S // 512           # 4 query blocks of 512
NKC = S // 128          # 16 key chunks
NPAIR = 2               # head pairs per core
SCALE = 1.0 / math.sqrt(DK)
NEG_BIAS = -30000.0     # exp(x + NEG_BIAS) == 0.0 in fp32 for any real score


def _r(ap):
    """Bitcast an fp32 AP to float32r so the PE runs at 1 cycle/row."""
    return ap.bitcast(F32R)


def _emit(nc, d):
    """Emit the per-core program.  d maps names -> DRAM tensor handles."""
    with tile.TileContext(nc) as tc, ExitStack() as top:
        consts = top.enter_context(tc.tile_pool(name="consts", bufs=1))
        persist = top.enter_context(tc.tile_pool(name="persist", bufs=1))

        # ---- tiles for constants / weights ----
        bq_sb = consts.tile([128, 2], F32, tag="bq", name="bqsb")
        bk_sb = consts.tile([128, 2], F32, tag="bk", name="bksb")
        nbias_sb = consts.tile([128, NKC], F32, tag="nbias", name="nbiassb")
        mdiag_f32 = consts.tile([128, 2, 128], F32, tag="mdf", name="mdiagf32")
        mdiag_sb = consts.tile([128, 2, 128], BF16, tag="mdiag", name="mdiagsb")
        swap_sb = consts.tile([128, 128], F32R, tag="swap", name="swapsb")
        wqq_sb = [consts.tile([128, NCH * 128], BF16, tag=f"wqq{p}",
                              name=f"wqq{p}sb") for p in range(NPAIR)]
        wkk_sb = [consts.tile([128, NCH * 128], BF16, tag=f"wkk{p}",
                              name=f"wkk{p}sb") for p in range(NPAIR)]
        wv_sb = consts.tile([128, NCH * 256], BF16, tag="wv", name="wvsb")
        wo_sb = consts.tile([128, 2 * 1024], F32R, tag="wo", name="wosb")
        xt_sb = [persist.tile([128, S], BF16, tag=f"xt{c}", name=f"xt{c}sb")
                 for c in range(NCH)]

        def dma_cols(sb, dram, lo, hi, pieces, bitcast=False, eng=None):
            eng = eng if eng is not None else nc.sync
            step_c = (hi - lo) // pieces
            for i in range(pieces):
                a, b_ = lo + i * step_c, lo + (i + 1) * step_c
                src = dram[:, a:b_]
                eng.dma_start(out=sb[:, a:b_],
                              in_=src.bitcast(F32R) if bitcast else src)

        # ---- input DMAs: weights on the SP DGE, x^T on the Activation DGE
        # (two hardware descriptor generators issue in parallel; ~0.6us per
        # dma_start on each sequencer is what paces the prologue) ----
        xtd = [d["xt"][c * 128:(c + 1) * 128, :] for c in range(NCH)]
        dma_cols(wqq_sb[0], d["wqq"][0], 0, 256, 2)
        dma_cols(xt_sb[0], xtd[0], 0, 512, 2, eng=nc.scalar)
        dma_cols(wqq_sb[0], d["wqq"][0], 256, 1024, 3)
        for c in range(1, NCH):
            dma_cols(xt_sb[c], xtd[c], 0, 512, 1, eng=nc.scalar)
        nc.sync.dma_start(out=bq_sb, in_=d["bq"][:])
        nc.sync.dma_start(out=bk_sb, in_=d["bk"][:])
        nc.sync.dma_start(out=nbias_sb, in_=d["nbias"][:])
        nc.sync.dma_start(out=mdiag_f32, in_=d["mdiag2"][:])
        nc.vector.tensor_copy(mdiag_sb, mdiag_f32)
        nc.sync.dma_start(out=swap_sb, in_=d["swap"][:].bitcast(F32R))
        dma_cols(wkk_sb[0], d["wkk"][0], 0, 1024, 4)
        dma_cols(wv_sb, d["wv"], 0, 2048, 4, eng=nc.scalar)
        dma_cols(wqq_sb[1], d["wqq"][1], 0, 1024, 4)
        dma_cols(wkk_sb[1], d["wkk"][1], 0, 1024, 4)
        for c in range(NCH):
            dma_cols(xt_sb[c], xtd[c], 512, 1024, 1)
        dma_cols(wo_sb, d["wo"], 0, 2048, 4, bitcast=True)
        for jb in (2, 3):
            for c in range(NCH):
                dma_cols(xt_sb[c], xtd[c], jb * 512, (jb + 1) * 512, 1)

        # ---- persistent activations ----
        qt_sb = []   # per pair: [128, S]; rows 0:64 head A Q^T, 64:128 head B
        kt_sb = []
        attnT = []   # per pair: [128, S]; normalized attn^T (dims on rows)
        for p in range(NPAIR):
            qt_sb.append(persist.tile([128, S], BF16, tag=f"qt{p}",
                                      name=f"qt{p}sb"))
            kt_sb.append(persist.tile([128, S], BF16, tag=f"kt{p}",
                                      name=f"kt{p}sb"))
            attnT.append(persist.tile([128, S], F32R, tag=f"at{p}",
                                      name=f"at{p}sb"))
        # V_aug [128, parity, pair, keycols]: even heads (A) = [V | ones],
        # odd heads (B) = [ones | V]
        vaug = persist.tile([128, 2, 2, NKC * 128], BF16, tag="vaug",
                            name="vaugsb")
        nc.gpsimd.memset(vaug, 1.0)

        # ---- PSUM pools: 4 (scores+proj) + 2 (outproj/den) + 2 (oA,oB) ----
        sp = top.enter_context(tc.tile_pool(name="sp", bufs=2, space="PSUM"))
        rot = top.enter_context(tc.tile_pool(name="rot", bufs=2, space="PSUM"))
        op = top.enter_context(tc.tile_pool(name="op", bufs=1, space="PSUM"))

        # ---- SBUF work pools ----
        ptp = top.enter_context(tc.tile_pool(name="ptp", bufs=6))
        nrm = top.enter_context(tc.tile_pool(name="nrm", bufs=3))
        ysb = top.enter_context(tc.tile_pool(name="ysb", bufs=4))

        def proj_units(jb, kinds="qkv"):
            """Q/K/V projection for query block jb: independent units."""
            jsl = slice(jb * 512, (jb + 1) * 512)
            units = []
            for p in range(NPAIR):
                for wsb, bsb, dst, nm in (
                    (wqq_sb[p], bq_sb, qt_sb[p], "q"),
                    (wkk_sb[p], bk_sb, kt_sb[p], "k"),
                ):
                    if nm not in kinds:
                        continue
                    def u(p=p, wsb=wsb, bsb=bsb, dst=dst, nm=nm, jsl=jsl, jb=jb):
                        ps = sp.tile([128, 2, 512], F32, tag="s",
                                     name=f"ps{nm}{p}{jb}")
                        for c in range(NCH):
                            nc.tensor.matmul(
                                ps[:, 0, :],
                                wsb[:, c * 128:(c + 1) * 128],
                                xt_sb[c][:, jsl],
                                start=(c == 0), stop=(c == NCH - 1),
                            )
                        nc.vector.tensor_scalar_add(dst[:, jsl], ps[:, 0, :],
                                                    bsb[:, p:p + 1])
                    units.append(u)
            if "v" not in kinds:
                return units
            for t in range(4 * jb, 4 * jb + 4):
                def u(t=t):
                    ps = sp.tile([128, 4, 128], F32, tag="s", name=f"psv{t}")
                    for c in range(NCH):
                        nc.tensor.matmul(
                            ps[:, 0:2, :],
                            xt_sb[c][:, t * 128:(t + 1) * 128],
                            wv_sb[:, c * 256:(c + 1) * 256],
                            start=(c == 0), stop=(c == NCH - 1),
                        )
                    # ps cols = [h0|h1|h2|h3] x 64; even heads' V to parity 0
                    # front half, odd heads' V to parity 1 back half.
                    nc.vector.tensor_copy(
                        vaug[:, 0, :, t * 128:t * 128 + 64], ps[:, 0:2, 0:64])
                    nc.vector.tensor_copy(
                        vaug[:, 1, :, t * 128 + 64:(t + 1) * 128],
                        ps[:, 0:2, 64:128])
                units.append(u)
            return units

        def psf_units(j, tail=False):
            """Output projection for query block j: 8 independent units."""
            units = []
            for q in range(4 * j, 4 * j + 4):
                for half in range(2):
                    def u(q=q, half=half, tail=tail):
                        pf = rot.tile([128, 512], F32, tag="r",
                                      name=f"pf{q}{half}")
                        for p in range(NPAIR):
                            nc.tensor.matmul(
                                pf,
                                _r(attnT[p][:, q * 128:(q + 1) * 128]),
                                _r(wo_sb[:, p * 1024 + half * 512:
                                         p * 1024 + half * 512 + 512]),
                                start=(p == 0), stop=(p == 1),
                            )
                        yt = ysb.tile([128, 512], BF16, tag="y",
                                      name=f"yt{q}{half}")
                        if tail:
                            nc.scalar.copy(yt, pf)
                            eng = nc.sync if (q + half) % 2 else nc.scalar
                            eng.dma_start(
                                out=d["y"][q * 128:(q + 1) * 128,
                                           half * 512:(half + 1) * 512],
                                in_=yt)
                        else:
                            nc.vector.tensor_copy(yt, pf)
                            for piece in range(2):
                                ysl = slice(piece * 256, (piece + 1) * 256)
                                nc.gpsimd.dma_start(
                                    out=d["y"][q * 128:(q + 1) * 128,
                                               half * 512 + piece * 256:
                                               half * 512 + (piece + 1) * 256],
                                    in_=yt[:, ysl])
                    units.append(u)
            return units

        def emit_scores(p, j, c):
            """Scores + exp (+ diag mask) for chunk c; returns attnV args."""
            t = c - 4 * j
            fo = 128 * t if t > 0 else 0
            w = 512 - fo
            qsl = slice(j * 512 + fo, (j + 1) * 512)
            s2 = sp.tile([128, 2, 512], F32, tag="s", name=f"s{p}{j}{c}")
            nc.tensor.matmul(s2[:, 0:1, :w],
                             kt_sb[p][0:64, c * 128:(c + 1) * 128],
                             qt_sb[p][0:64, qsl], start=True, stop=True)
            nc.tensor.matmul(s2[:, 1:2, :w],
                             kt_sb[p][64:128, c * 128:(c + 1) * 128],
                             qt_sb[p][64:128, qsl], start=True, stop=True)
            p2 = ptp.tile([128, 2, 512], BF16, tag="p", name=f"p{p}{j}{c}")
            nc.scalar.activation(p2[:, :, :w], s2[:, :, :w], EXP,
                                 bias=nbias_sb[:, c:c + 1], scale=SCALE)
            if t >= 0:
                # diagonal 128x128 block: zero keys below the diagonal for
                # both heads in one op
                nc.vector.tensor_mul(p2[:, :, 0:128], p2[:, :, 0:128],
                                     mdiag_sb)
            return p2, fo, w

        def emit_attnv(p, j, c, oA, oB, cmax, p2, fo, w):
            ksl = slice(c * 128, (c + 1) * 128)
            nc.tensor.matmul(oA[:, fo:512], vaug[:, 0, p, ksl],
                             p2[:, 0:1, :w], start=(c == 0), stop=(c == cmax))
            nc.tensor.matmul(oB[:, fo:512], vaug[:, 1, p, ksl],
                             p2[:, 1:2, :w], start=(c == 0), stop=(c == cmax))

        def emit_norm(p, j, oA, oB):
            # denA = oA[64:128], denB = oB[0:64]; swap halves on the PE so
            # each reciprocal lands on its numerator's partitions.
            jsl = slice(j * 512, (j + 1) * 512)
            scr = nrm.tile([128, 512], F32R, tag="scr", name=f"scr{p}{j}")
            nc.vector.tensor_copy(scr[64:128, :], oA[64:128, :])
            nc.vector.tensor_copy(scr[0:64, :], oB[0:64, :])
            den2 = rot.tile([128, 512], F32, tag="r", name=f"den{p}{j}")
            nc.tensor.matmul(den2, swap_sb, scr, start=True, stop=True)
            rec = nrm.tile([128, 512], F32, tag="rec", name=f"rec{p}{j}")
            nc.vector.reciprocal_approx_fast(out=rec, in_=den2)
            nc.vector.tensor_mul(attnT[p][0:64, jsl], oA[0:64, :],
                                 rec[0:64, :])
            nc.vector.tensor_mul(attnT[p][64:128, jsl], oB[64:128, :],
                                 rec[64:128, :])

        # ---- main schedule ----
        for u in proj_units(0):
            u()
        # Filler assignment keeps every step PE-bound.  Block 3's K and V
        # projections are only consumed from chunk 12 of step 3, so they
        # slide into step 3 itself as guaranteed-ready PE filler for its
        # exp-heavy stretch; out-projections lag two steps for the same
        # reason.
        for j in range(NJ):
            if j == 0:
                fillers = proj_units(1)
            elif j == 1:
                fillers = proj_units(2) + psf_units(0)
            elif j == 2:
                fillers = proj_units(3, kinds="q") + psf_units(1)
            else:
                fillers = proj_units(3, kinds="kv") + psf_units(2)
            nch_j = 4 * j + 4
            total_chunks = 2 * nch_j
            done = 0
            ci = 0
            for p in range(NPAIR):
                oA = op.tile([128, 512], F32, tag="oA", name=f"oA{p}{j}")
                oB = op.tile([128, 512], F32, tag="oB", name=f"oB{p}{j}")
                pend = []
                for c in range(nch_j):
                    pend.append((c,) + emit_scores(p, j, c))
                    if len(pend) > 2:
                        c0, p2, fo, w = pend.pop(0)
                        emit_attnv(p, j, c0, oA, oB, nch_j - 1, p2, fo, w)
                    ci += 1
                    want = ci * len(fillers) // total_chunks
                    while done < want:
                        fillers[done]()
                        done += 1
                for c0, p2, fo, w in pend:
                    emit_attnv(p, j, c0, oA, oB, nch_j - 1, p2, fo, w)
                emit_norm(p, j, oA, oB)
            while done < len(fillers):
                fillers[done]()
                done += 1
        for u in psf_units(NJ - 1, tail=True):
            u()

        if _DEBUG:
            for p in range(NPAIR):
                nc.sync.dma_start(out=d[f"dbg_qt{p}"][:], in_=qt_sb[p].bitcast(F32))
                nc.sync.dma_start(out=d[f"dbg_kt{p}"][:], in_=kt_sb[p].bitcast(F32))
                nc.sync.dma_start(out=d[f"dbg_at{p}"][:], in_=attnT[p].bitcast(F32))
            for h in range(4):
                nc.sync.dma_start(out=d[f"dbg_va{h}"][:], in_=vaug[h].bitcast(F32))


_NC_CACHE = {}
_DEBUG = False


def _get_nc():
    if "nc" not in _NC_CACHE:
        nc = bacc.Bacc(None, target_bir_lowering=False)
        d = {
            "xt": nc.dram_tensor("xt", [H, S], BF16, kind="ExternalInput"),
            "wqq": nc.dram_tensor("wqq", [NPAIR, 128, NCH * 128], BF16,
                                  kind="ExternalInput"),
            "wkk": nc.dram_tensor("wkk", [NPAIR, 128, NCH * 128], BF16,
                                  kind="ExternalInput"),
            "wv": nc.dram_tensor("wv", [128, NCH * 256], BF16, kind="ExternalInput"),
            "wo": nc.dram_tensor("wo", [128, 2 * 1024], F32, kind="ExternalInput"),
            "bq": nc.dram_tensor("bq", [128, 2], F32, kind="ExternalInput"),
            "bk": nc.dram_tensor("bk", [128, 2], F32, kind="ExternalInput"),
            "nbias": nc.dram_tensor("nbias", [128, NKC], F32, kind="ExternalInput"),
            "mdiag2": nc.dram_tensor("mdiag2", [128, 2, 128], F32,
                                     kind="ExternalInput"),
            "swap": nc.dram_tensor("swap", [128, 128], F32, kind="ExternalInput"),
            "y": nc.dram_tensor("y", [S, H], BF16, kind="ExternalOutput"),
        }
        if _DEBUG:
            for p in range(NPAIR):
                d[f"dbg_qt{p}"] = nc.dram_tensor(f"dbg_qt{p}", [128, S], F32,
                                                 kind="ExternalOutput")
                d[f"dbg_kt{p}"] = nc.dram_tensor(f"dbg_kt{p}", [128, S], F32,
                                                 kind="ExternalOutput")
                d[f"dbg_at{p}"] = nc.dram_tensor(f"dbg_at{p}", [128, S], F32,
                                                 kind="ExternalOutput")
            for h in range(4):
                d[f"dbg_va{h}"] = nc.dram_tensor(f"dbg_va{h}", [128, NKC * 128],
                                                 F32, kind="ExternalOutput")
        _emit(nc, d)
        nc.finalize()
        _NC_CACHE["nc"] = nc
    return _NC_CACHE["nc"]


def _chunked(w, ncols):
    """[H, ncols] -> [128, NCH*ncols] with chunk c of rows at cols c*ncols."""
    return np.ascontiguousarray(
        w.reshape(NCH, 128, ncols).transpose(1, 0, 2).reshape(128, NCH * ncols))


def _make_in_maps(batch, input_ids, W_Q, W_K, W_V, W_O, b_Q, b_K):
    m = np.triu(np.ones((128, 128), np.float32))
    mdiag2 = np.ascontiguousarray(np.stack([m, m], axis=1))  # [128, 2, 128]
    swap = np.zeros((128, 128), np.float32)
    swap[64:128, 0:64] = np.eye(64, dtype=np.float32)
    swap[0:64, 64:128] = np.eye(64, dtype=np.float32)
    in_maps = []
    for core in range(NCORE):
        b, g = divmod(core, 4)
        base = 256 * g  # first feature column of this core's 4 heads
        wqq = np.stack([_chunked(W_Q[:, base + 128 * p: base + 128 * (p + 1)], 128)
                        for p in range(NPAIR)])
        wkk = np.stack([_chunked(W_K[:, base + 128 * p: base + 128 * (p + 1)], 128)
                        for p in range(NPAIR)])
        wv = _chunked(W_V[:, base: base + 256], 256)
        wo = np.ascontiguousarray(
            W_O[base: base + 256, :].reshape(2, 128, H)
            .transpose(1, 0, 2).reshape(128, 2 * H))
        bq = np.stack([b_Q[base + 128 * p: base + 128 * (p + 1)]
                       for p in range(NPAIR)], axis=1)
        bk = np.stack([b_K[base + 128 * p: base + 128 * (p + 1)]
                       for p in range(NPAIR)], axis=1)
        keep = input_ids[b] != 0
        nbias = np.where(keep, 0.0, NEG_BIAS).astype(np.float32)
        nbias = np.ascontiguousarray(nbias.reshape(NKC, 128).T)
        xt = np.ascontiguousarray(batch[b].T)
        bf = ml_dtypes.bfloat16
        in_maps.append({
            "xt": xt.astype(bf), "wqq": wqq.astype(bf),
            "wkk": wkk.astype(bf), "wv": wv.astype(bf), "wo": wo,
            "bq": np.ascontiguousarray(bq), "bk": np.ascontiguousarray(bk),
            "nbias": nbias, "mdiag2": mdiag2, "swap": swap,
        })
    return in_maps


def _run(in_maps, **kwargs):
    nc = _get_nc()
    return run_bass_kernel_spmd(nc, in_maps, core_ids=list(range(NCORE)), **kwargs)


def kernel(batch, input_ids, W_Q, W_K, W_V, b_Q, b_K, b_V, W_O, b_O,
           _results_out=None, **run_kwargs):
    batch = np.asarray(batch, np.float32)
    input_ids = np.asarray(input_ids)
    W_Q, W_K, W_V = (np.asarray(a, np.float32) for a in (W_Q, W_K, W_V))
    b_Q, b_K, b_V = (np.asarray(a, np.float32) for a in (b_Q, b_K, b_V))
    W_O = np.asarray(W_O, np.float32)
    b_O = np.asarray(b_O, np.float32)

    in_maps = _make_in_maps(batch, input_ids, W_Q, W_K, W_V, W_O, b_Q, b_K)
    res = _run(in_maps, **run_kwargs)
    if _results_out is not None:
        _results_out.append(res)
    ys = [np.asarray(res.results[c]["y"], np.float32) for c in range(NCORE)]
    out = np.stack([sum(ys[4 * b: 4 * b + 4]) for b in range(B)], axis=0)
    # b_V enters as attn@1 * b_V = b_V (softmax rows sum to 1), then @ W_O.
    const_row = (b_V @ W_O + b_O).astype(np.float32)
    return (out + const_row).astype(np.float32)


# revision 15
# speedup vs baseline: 1.0194x; 1.0097x over previous
"""Multi-head attention (B=2, S=2048, H=1024, NH=16, DK=DV=64) on 8 TRN2 cores.

Sharding: data-parallel over batch (2 groups of 4 cores) x tensor-parallel
over heads (4 heads per core, as 2 pairs of 2).  Each core computes, for its
batch sample and its 4 heads:

    Q^T/K^T projections (features on partitions), V projection (natural),
    S^T = K @ Q^T per 128-key chunk (causal chunks only, 2 heads fused into
    one 2-bank PSUM tile),
    P^T = exp(S^T/8 + pad_bias)   (one fused exp over both heads; no
    max-subtraction needed: |scores| ~ N(0,1)),
    out^T = V_aug^T @ P^T  where V_aug = [V | ones] for head A and
    [ones | V] for head B, so oA = [attnA; denA] and oB = [denB; attnB].
    A single PE matmul with a half-swap permutation aligns both
    denominators with their numerators; one reciprocal + two muls
    normalize directly into attnT.
    y_partial = attnT^T @ W_O_rows   (row-sharded W_O).

Host sums the 4 partials per batch and adds (b_V @ W_O + b_O) (exact since
softmax rows sum to 1).

Pipelining: x^T is DMA'd in query-column blocks (split into <=256KB pieces
so no single DMA queue gates the PE start); projections for query block
jb+1 and the output projection for block j-1 are emitted interleaved into
block j's attention chunk stream so the PE never idles while the
activation engine works through the exps.  PSUM: 4 banks score/proj
rotation + 2 banks out-proj/denominator rotation + 2 banks attnV
accumulators = 8.
"""

import math
from contextlib import ExitStack

import ml_dtypes
import numpy as np


import concourse.bass as bass
import concourse.mybir as mybir
from concourse import bacc
import concourse.tile as tile
from concourse.bass_utils import run_bass_kernel_spmd

F32 = mybir.dt.float32
F32R = mybir.dt.float32r
BF16 = mybir.dt.bfloat16
EXP = mybir.ActivationFunctionType.Exp

B, S, H = 2, 2048, 1024
NH, DK, DV = 16, 64, 64
NCORE = 8
NCH = H // 128          # 8 contraction chunks over H
NJ = S // 512           # 4 query blocks of 512
NKC = S // 128          # 16 key chunks
NPAIR = 2               # head pairs per core
SCALE = 1.0 / math.sqrt(DK)
NEG_BIAS = -30000.0     # exp(x + NEG_BIAS) == 0.0 in fp32 for any real score


def _r(ap):
    """Bitcast an fp32 AP to float32r so the PE runs at 1 cycle/row."""
    return ap.bitcast(F32R)


def _emit(nc, d):
    """Emit the per-core program.  d maps names -> DRAM tensor handles."""
    with tile.TileContext(nc) as tc, ExitStack() as top:
        consts = top.enter_context(tc.tile_pool(name="consts", bufs=1))
        persist = top.enter_context(tc.tile_pool(name="persist", bufs=1))

        # ---- tiles for constants / weights ----
        bq_sb = consts.tile([128, 2], F32, tag="bq", name="bqsb")
        bk_sb = consts.tile([128, 2], F32, tag="bk", name="bksb")
        nbias_sb = consts.tile([128, NKC], F32, tag="nbias", name="nbiassb")
        mdiag_f32 = consts.tile([128, 2, 128], F32, tag="mdf", name="mdiagf32")
        mdiag_sb = consts.tile([128, 2, 128], BF16, tag="mdiag", name="mdiagsb")
        swap_sb = consts.tile([128, 128], F32R, tag="swap", name="swapsb")
        wqq_sb = [consts.tile([128, NCH * 128], BF16, tag=f"wqq{p}",
                              name=f"wqq{p}sb") for p in range(NPAIR)]
        wkk_sb = [consts.tile([128, NCH * 128], BF16, tag=f"wkk{p}",
                              name=f"wkk{p}sb") for p in range(NPAIR)]
        wv_sb = consts.tile([128, NCH * 256], BF16, tag="wv", name="wvsb")
        wo_sb = consts.tile([128, 2 * 1024], F32R, tag="wo", name="wosb")
        xt_sb = [persist.tile([128, S], BF16, tag=f"xt{c}", name=f"xt{c}sb")
                 for c in range(NCH)]

        def dma_cols(sb, dram, lo, hi, pieces, bitcast=False, eng=None):
            eng = eng if eng is not None else nc.sync
            step_c = (hi - lo) // pieces
            for i in range(pieces):
                a, b_ = lo + i * step_c, lo + (i + 1) * step_c
                src = dram[:, a:b_]
                eng.dma_start(out=sb[:, a:b_],
                              in_=src.bitcast(F32R) if bitcast else src)

        # ---- input DMAs: weights on the SP DGE, x^T on the Activation DGE
        # (two hardware descriptor generators issue in parallel; ~0.6us per
        # dma_start on each sequencer is what paces the prologue) ----
        xtd = [d["xt"][c * 128:(c + 1) * 128, :] for c in range(NCH)]
        dma_cols(wqq_sb[0], d["wqq"][0], 0, 256, 2)
        dma_cols(xt_sb[0], xtd[0], 0, 512, 2, eng=nc.scalar)
        dma_cols(wqq_sb[0], d["wqq"][0], 256, 1024, 3)
        for c in range(1, NCH):
            dma_cols(xt_sb[c], xtd[c], 0, 512, 1, eng=nc.scalar)
        nc.sync.dma_start(out=bq_sb, in_=d["bq"][:])
        nc.sync.dma_start(out=bk_sb, in_=d["bk"][:])
        nc.sync.dma_start(out=nbias_sb, in_=d["nbias"][:])
        nc.sync.dma_start(out=mdiag_f32, in_=d["mdiag2"][:])
        nc.vector.tensor_copy(mdiag_sb, mdiag_f32)
        nc.sync.dma_start(out=swap_sb, in_=d["swap"][:].bitcast(F32R))
        dma_cols(wkk_sb[0], d["wkk"][0], 0, 1024, 4)
        dma_cols(wv_sb, d["wv"], 0, 2048, 4, eng=nc.scalar)
        dma_cols(wqq_sb[1], d["wqq"][1], 0, 1024, 4)
        dma_cols(wkk_sb[1], d["wkk"][1], 0, 1024, 4)
        for c in range(NCH):
            dma_cols(xt_sb[c], xtd[c], 512, 1024, 1)
        dma_cols(wo_sb, d["wo"], 0, 2048, 4, bitcast=True)
        for jb in (2, 3):
            for c in range(NCH):
                dma_cols(xt_sb[c], xtd[c], jb * 512, (jb + 1) * 512, 1)

        # ---- persistent activations ----
        qt_sb = []   # per pair: [128, S]; rows 0:64 head A Q^T, 64:128 head B
        kt_sb = []
        attnT = []   # per pair: [128, S]; normalized attn^T (dims on rows)
        for p in range(NPAIR):
            qt_sb.append(persist.tile([128, S], BF16, tag=f"qt{p}",
                                      name=f"qt{p}sb"))
            kt_sb.append(persist.tile([128, S], BF16, tag=f"kt{p}",
                                      name=f"kt{p}sb"))
            attnT.append(persist.tile([128, S], F32R, tag=f"at{p}",
                                      name=f"at{p}sb"))
        # V_aug [128, parity, pair, keycols]: even heads (A) = [V | ones],
        # odd heads (B) = [ones | V]
        vaug = persist.tile([128, 2, 2, NKC * 128], BF16, tag="vaug",
                            name="vaugsb")
        nc.gpsimd.memset(vaug, 1.0)

        # ---- PSUM pools: 4 (scores+proj) + 2 (outproj/den) + 2 (oA,oB) ----
        sp = top.enter_context(tc.tile_pool(name="sp", bufs=2, space="PSUM"))
        rot = top.enter_context(tc.tile_pool(name="rot", bufs=2, space="PSUM"))
        op = top.enter_context(tc.tile_pool(name="op", bufs=1, space="PSUM"))

        # ---- SBUF work pools ----
        ptp = top.enter_context(tc.tile_pool(name="ptp", bufs=6))
        nrm = top.enter_context(tc.tile_pool(name="nrm", bufs=3))
        ysb = top.enter_context(tc.tile_pool(name="ysb", bufs=4))

        def proj_units(jb, kinds="qkv"):
            """Q/K/V projection for query block jb: independent units."""
            jsl = slice(jb * 512, (jb + 1) * 512)
            units = []
            for p in range(NPAIR):
                for wsb, bsb, dst, nm in (
                    (wqq_sb[p], bq_sb, qt_sb[p], "q"),
                    (wkk_sb[p], bk_sb, kt_sb[p], "k"),
                ):
                    if nm not in kinds:
                        continue
                    def u(p=p, wsb=wsb, bsb=bsb, dst=dst, nm=nm, jsl=jsl, jb=jb):
                        ps = sp.tile([128, 2, 512], F32, tag="s",
                                     name=f"ps{nm}{p}{jb}")
                        for c in range(NCH):
                            nc.tensor.matmul(
                                ps[:, 0, :],
                                wsb[:, c * 128:(c + 1) * 128],
                                xt_sb[c][:, jsl],
                                start=(c == 0), stop=(c == NCH - 1),
                            )
                        nc.vector.tensor_scalar_add(dst[:, jsl], ps[:, 0, :],
                                                    bsb[:, p:p + 1])
                    units.append(u)
            if "v" not in kinds:
                return units
            for t in range(4 * jb, 4 * jb + 4):
                def u(t=t):
                    ps = sp.tile([128, 4, 128], F32, tag="s", name=f"psv{t}")
                    for c in range(NCH):
                        nc.tensor.matmul(
                            ps[:, 0:2, :],
                            xt_sb[c][:, t * 128:(t + 1) * 128],
                            wv_sb[:, c * 256:(c + 1) * 256],
                            start=(c == 0), stop=(c == NCH - 1),
                        )
                    # ps cols = [h0|h1|h2|h3] x 64; even heads' V to parity 0
                    # front half, odd heads' V to parity 1 back half.
                    nc.vector.tensor_copy(
                        vaug[:, 0, :, t * 128:t * 128 + 64], ps[:, 0:2, 0:64])
                    nc.vector.tensor_copy(
                        vaug[:, 1, :, t * 128 + 64:(t + 1) * 128],
                        ps[:, 0:2, 64:128])
                units.append(u)
            return units

        def psf_units(j, tail=False):
            """Output projection for query block j: 8 independent units."""
            units = []
            for q in range(4 * j, 4 * j + 4):
                for half in range(2):
                    def u(q=q, half=half, tail=tail):
                        pf = rot.tile([128, 512], F32, tag="r",
                                      name=f"pf{q}{half}")
                        for p in range(NPAIR):
                            nc.tensor.matmul(
                                pf,
                                _r(attnT[p][:, q * 128:(q + 1) * 128]),
                                _r(wo_sb[:, p * 1024 + half * 512:
                                         p * 1024 + half * 512 + 512]),
                                start=(p == 0), stop=(p == 1),
                            )
                        yt = ysb.tile([128, 512], BF16, tag="y",
                                      name=f"yt{q}{half}")
                        if tail:
                            nc.scalar.copy(yt, pf)
                            eng = nc.sync if (q + half) % 2 else nc.scalar
                            eng.dma_start(
                                out=d["y"][q * 128:(q + 1) * 128,
                                           half * 512:(half + 1) * 512],
                                in_=yt)
                        else:
                            nc.vector.tensor_copy(yt, pf)
                            for piece in range(2):
                                ysl = slice(piece * 256, (piece + 1) * 256)
                                nc.gpsimd.dma_start(
                                    out=d["y"][q * 128:(q + 1) * 128,
                                               half * 512 + piece * 256:
                                               half * 512 + (piece + 1) * 256],
                                    in_=yt[:, ysl])
                    units.append(u)
            return units

        def emit_scores(p, j, c):
            """Scores + exp (+ diag mask) for chunk c; returns attnV args."""
            t = c - 4 * j
            fo = 128 * t if t > 0 else 0
            w = 512 - fo
            qsl = slice(j * 512 + fo, (j + 1) * 512)
            with tc.high_priority(offset=40):
                s2 = sp.tile([128, 2, 512], F32, tag="s", name=f"s{p}{j}{c}")
                nc.tensor.matmul(s2[:, 0:1, :w],
                                 kt_sb[p][0:64, c * 128:(c + 1) * 128],
                                 qt_sb[p][0:64, qsl], start=True, stop=True)
                nc.tensor.matmul(s2[:, 1:2, :w],
                                 kt_sb[p][64:128, c * 128:(c + 1) * 128],
                                 qt_sb[p][64:128, qsl], start=True, stop=True)
                p2 = ptp.tile([128, 2, 512], BF16, tag="p", name=f"p{p}{j}{c}")
                nc.scalar.activation(p2[:, :, :w], s2[:, :, :w], EXP,
                                     bias=nbias_sb[:, c:c + 1], scale=SCALE)
            if t >= 0:
                # diagonal 128x128 block: zero keys below the diagonal for
                # both heads in one op
                nc.vector.tensor_mul(p2[:, :, 0:128], p2[:, :, 0:128],
                                     mdiag_sb)
            return p2, fo, w

        def emit_attnv(p, j, c, oA, oB, cmax, p2, fo, w):
            ksl = slice(c * 128, (c + 1) * 128)
            nc.tensor.matmul(oA[:, fo:512], vaug[:, 0, p, ksl],
                             p2[:, 0:1, :w], start=(c == 0), stop=(c == cmax))
            nc.tensor.matmul(oB[:, fo:512], vaug[:, 1, p, ksl],
                             p2[:, 1:2, :w], start=(c == 0), stop=(c == cmax))

        def emit_norm(p, j, oA, oB):
            # denA = oA[64:128], denB = oB[0:64]; swap halves on the PE so
            # each reciprocal lands on its numerator's partitions.
            jsl = slice(j * 512, (j + 1) * 512)
            scr = nrm.tile([128, 512], F32R, tag="scr", name=f"scr{p}{j}")
            nc.vector.tensor_copy(scr[64:128, :], oA[64:128, :])
            nc.vector.tensor_copy(scr[0:64, :], oB[0:64, :])
            den2 = rot.tile([128, 512], F32, tag="r", name=f"den{p}{j}")
            nc.tensor.matmul(den2, swap_sb, scr, start=True, stop=True)
            rec = nrm.tile([128, 512], F32, tag="rec", name=f"rec{p}{j}")
            nc.vector.reciprocal_approx_fast(out=rec, in_=den2)
            nc.vector.tensor_mul(attnT[p][0:64, jsl], oA[0:64, :],
                                 rec[0:64, :])
            nc.vector.tensor_mul(attnT[p][64:128, jsl], oB[64:128, :],
                                 rec[64:128, :])

        # ---- main schedule ----
        for u in proj_units(0):
            u()
        # Filler assignment keeps every step PE-bound.  Block 3's K and V
        # projections are only consumed from chunk 12 of step 3, so they
        # slide into step 3 itself as guaranteed-ready PE filler for its
        # exp-heavy stretch; out-projections lag two steps for the same
        # reason.
        for j in range(NJ):
            if j == 0:
                fillers = proj_units(1)
            elif j == 1:
                fillers = proj_units(2) + psf_units(0)
            elif j == 2:
                fillers = proj_units(3, kinds="q") + psf_units(1)
            else:
                fillers = proj_units(3, kinds="kv") + psf_units(2)
            nch_j = 4 * j + 4
            total_chunks = 2 * nch_j
            done = 0
            ci = 0
            for p in range(NPAIR):
                oA = op.tile([128, 512], F32, tag="oA", name=f"oA{p}{j}")
                oB = op.tile([128, 512], F32, tag="oB", name=f"oB{p}{j}")
                pend = []
                for c in range(nch_j):
                    pend.append((c,) + emit_scores(p, j, c))
                    if len(pend) > 2:
                        c0, p2, fo, w = pend.pop(0)
                        emit_attnv(p, j, c0, oA, oB, nch_j - 1, p2, fo, w)
                    ci += 1
                    want = ci * len(fillers) // total_chunks
                    while done < want:
                        fillers[done]()
                        done += 1
                for c0, p2, fo, w in pend:
                    emit_attnv(p, j, c0, oA, oB, nch_j - 1, p2, fo, w)
                emit_norm(p, j, oA, oB)
            while done < len(fillers):
                fillers[done]()
                done += 1
        for u in psf_units(NJ - 1, tail=True):
            u()

        if _DEBUG:
            for p in range(NPAIR):
                nc.sync.dma_start(out=d[f"dbg_qt{p}"][:], in_=qt_sb[p].bitcast(F32))
                nc.sync.dma_start(out=d[f"dbg_kt{p}"][:], in_=kt_sb[p].bitcast(F32))
                nc.sync.dma_start(out=d[f"dbg_at{p}"][:], in_=attnT[p].bitcast(F32))
            for h in range(4):
                nc.sync.dma_start(out=d[f"dbg_va{h}"][:], in_=vaug[h].bitcast(F32))


_NC_CACHE = {}
_DEBUG = False


def _get_nc():
    if "nc" not in _NC_CACHE:
        nc = bacc.Bacc(None, target_bir_lowering=False)
        d = {
            "xt": nc.dram_tensor("xt", [H, S], BF16, kind="ExternalInput"),
            "wqq": nc.dram_tensor("wqq", [NPAIR, 128, NCH * 128], BF16,
                                  kind="ExternalInput"),
            "wkk": nc.dram_tensor("wkk", [NPAIR, 128, NCH * 128], BF16,
                                  kind="ExternalInput"),
            "wv": nc.dram_tensor("wv", [128, NCH * 256], BF16, kind="ExternalInput"),
            "wo": nc.dram_tensor("wo", [128, 2 * 1024], F32, kind="ExternalInput"),
            "bq": nc.dram_tensor("bq", [128, 2], F32, kind="ExternalInput"),
            "bk": nc.dram_tensor("bk", [128, 2], F32, kind="ExternalInput"),
            "nbias": nc.dram_tensor("nbias", [128, NKC], F32, kind="ExternalInput"),
            "mdiag2": nc.dram_tensor("mdiag2", [128, 2, 128], F32,
                                     kind="ExternalInput"),
            "swap": nc.dram_tensor("swap", [128, 128], F32, kind="ExternalInput"),
            "y": nc.dram_tensor("y", [S, H], BF16, kind="ExternalOutput"),
        }
        if _DEBUG:
            for p in range(NPAIR):
                d[f"dbg_qt{p}"] = nc.dram_tensor(f"dbg_qt{p}", [128, S], F32,
                                                 kind="ExternalOutput")
                d[f"dbg_kt{p}"] = nc.dram_tensor(f"dbg_kt{p}", [128, S], F32,
                                                 kind="ExternalOutput")
                d[f"dbg_at{p}"] = nc.dram_tensor(f"dbg_at{p}", [128, S], F32,
                                                 kind="ExternalOutput")
            for h in range(4):
                d[f"dbg_va{h}"] = nc.dram_tensor(f"dbg_va{h}", [128, NKC * 128],
                                                 F32, kind="ExternalOutput")
        _emit(nc, d)
        nc.finalize()
        _NC_CACHE["nc"] = nc
    return _NC_CACHE["nc"]


def _chunked(w, ncols):
    """[H, ncols] -> [128, NCH*ncols] with chunk c of rows at cols c*ncols."""
    return np.ascontiguousarray(
        w.reshape(NCH, 128, ncols).transpose(1, 0, 2).reshape(128, NCH * ncols))


def _make_in_maps(batch, input_ids, W_Q, W_K, W_V, W_O, b_Q, b_K):
    m = np.triu(np.ones((128, 128), np.float32))
    mdiag2 = np.ascontiguousarray(np.stack([m, m], axis=1))  # [128, 2, 128]
    swap = np.zeros((128, 128), np.float32)
    swap[64:128, 0:64] = np.eye(64, dtype=np.float32)
    swap[0:64, 64:128] = np.eye(64, dtype=np.float32)
    in_maps = []
    for core in range(NCORE):
        b, g = divmod(core, 4)
        base = 256 * g  # first feature column of this core's 4 heads
        wqq = np.stack([_chunked(W_Q[:, base + 128 * p: base + 128 * (p + 1)], 128)
                        for p in range(NPAIR)])
        wkk = np.stack([_chunked(W_K[:, base + 128 * p: base + 128 * (p + 1)], 128)
                        for p in range(NPAIR)])
        wv = _chunked(W_V[:, base: base + 256], 256)
        wo = np.ascontiguousarray(
            W_O[base: base + 256, :].reshape(2, 128, H)
            .transpose(1, 0, 2).reshape(128, 2 * H))
        bq = np.stack([b_Q[base + 128 * p: base + 128 * (p + 1)]
                       for p in range(NPAIR)], axis=1)
        bk = np.stack([b_K[base + 128 * p: base + 128 * (p + 1)]
                       for p in range(NPAIR)], axis=1)
        keep = input_ids[b] != 0
        nbias = np.where(keep, 0.0, NEG_BIAS).astype(np.float32)
        nbias = np.ascontiguousarray(nbias.reshape(NKC, 128).T)
        xt = np.ascontiguousarray(batch[b].T)
        bf = ml_dtypes.bfloat16
        in_maps.append({
            "xt": xt.astype(bf), "wqq": wqq.astype(bf),
            "wkk": wkk.astype(bf), "wv": wv.astype(bf), "wo": wo,
            "bq": np.ascontiguousarray(bq), "bk": np.ascontiguousarray(bk),
            "nbias": nbias, "mdiag2": mdiag2, "swap": swap,
        })
    return in_maps


def _run(in_maps, **kwargs):
    nc = _get_nc()
    return run_bass_kernel_spmd(nc, in_maps, core_ids=list(range(NCORE)), **kwargs)


def kernel(batch, input_ids, W_Q, W_K, W_V, b_Q, b_K, b_V, W_O, b_O,
           _results_out=None, **run_kwargs):
    batch = np.asarray(batch, np.float32)
    input_ids = np.asarray(input_ids)
    W_Q, W_K, W_V = (np.asarray(a, np.float32) for a in (W_Q, W_K, W_V))
    b_Q, b_K, b_V = (np.asarray(a, np.float32) for a in (b_Q, b_K, b_V))
    W_O = np.asarray(W_O, np.float32)
    b_O = np.asarray(b_O, np.float32)

    in_maps = _make_in_maps(batch, input_ids, W_Q, W_K, W_V, W_O, b_Q, b_K)
    res = _run(in_maps, **run_kwargs)
    if _results_out is not None:
        _results_out.append(res)
    ys = [np.asarray(res.results[c]["y"], np.float32) for c in range(NCORE)]
    out = np.stack([sum(ys[4 * b: 4 * b + 4]) for b in range(B)], axis=0)
    # b_V enters as attn@1 * b_V = b_V (softmax rows sum to 1), then @ W_O.
    const_row = (b_V @ W_O + b_O).astype(np.float32)
    return (out + const_row).astype(np.float32)


# revision 16
# speedup vs baseline: 1.0289x; 1.0093x over previous
"""Multi-head attention (B=2, S=2048, H=1024, NH=16, DK=DV=64) on 8 TRN2 cores.

Sharding: data-parallel over batch (2 groups of 4 cores) x tensor-parallel
over heads (4 heads per core, as 2 pairs of 2).  Each core computes, for its
batch sample and its 4 heads:

    Q^T/K^T projections (features on partitions), V projection (natural),
    S^T = K @ Q^T per 128-key chunk (causal chunks only, 2 heads fused into
    one 2-bank PSUM tile),
    P^T = exp(S^T/8 + pad_bias)   (one fused exp over both heads; no
    max-subtraction needed: |scores| ~ N(0,1)),
    out^T = V_aug^T @ P^T  where V_aug = [V | ones] for head A and
    [ones | V] for head B, so oA = [attnA; denA] and oB = [denB; attnB].
    A single PE matmul with a half-swap permutation aligns both
    denominators with their numerators; one reciprocal + two muls
    normalize directly into attnT.
    y_partial = attnT^T @ W_O_rows   (row-sharded W_O).

Host sums the 4 partials per batch and adds (b_V @ W_O + b_O) (exact since
softmax rows sum to 1).

Pipelining: x^T is DMA'd in query-column blocks (split into <=256KB pieces
so no single DMA queue gates the PE start); projections for query block
jb+1 and the output projection for block j-1 are emitted interleaved into
block j's attention chunk stream so the PE never idles while the
activation engine works through the exps.  PSUM: 4 banks score/proj
rotation + 2 banks out-proj/denominator rotation + 2 banks attnV
accumulators = 8.
"""

import math
from contextlib import ExitStack

import ml_dtypes
import numpy as np


import concourse.bass as bass
import concourse.mybir as mybir
from concourse import bacc
import concourse.tile as tile
from concourse.bass_utils import run_bass_kernel_spmd

F32 = mybir.dt.float32
F32R = mybir.dt.float32r
BF16 = mybir.dt.bfloat16
EXP = mybir.ActivationFunctionType.Exp

B, S, H = 2, 2048, 1024
NH, DK, DV = 16, 64, 64
NCORE = 8
NCH = H // 128          # 8 contraction chunks over H
NJ = S // 512           # 4 query blocks of 512
NKC = S // 128          # 16 key chunks
NPAIR = 2               # head pairs per core
SCALE = 1.0 / math.sqrt(DK)
NEG_BIAS = -30000.0     # exp(x + NEG_BIAS) == 0.0 in fp32 for any real score


def _r(ap):
    """Bitcast an fp32 AP to float32r so the PE runs at 1 cycle/row."""
    return ap.bitcast(F32R)


def _emit(nc, d):
    """Emit the per-core program.  d maps names -> DRAM tensor handles."""
    with tile.TileContext(nc) as tc, ExitStack() as top:
        consts = top.enter_context(tc.tile_pool(name="consts", bufs=1))
        persist = top.enter_context(tc.tile_pool(name="persist", bufs=1))

        # ---- tiles for constants / weights ----
        bq_sb = consts.tile([128, 2], F32, tag="bq", name="bqsb")
        bk_sb = consts.tile([128, 2], F32, tag="bk", name="bksb")
        nbias_sb = consts.tile([128, NKC], F32, tag="nbias", name="nbiassb")
        mdiag_f32 = consts.tile([128, 2, 128], F32, tag="mdf", name="mdiagf32")
        mdiag_sb = consts.tile([128, 2, 128], BF16, tag="mdiag", name="mdiagsb")
        swap_sb = consts.tile([128, 128], F32R, tag="swap", name="swapsb")
        wqq_sb = [consts.tile([128, NCH * 128], BF16, tag=f"wqq{p}",
                              name=f"wqq{p}sb") for p in range(NPAIR)]
        wkk_sb = [consts.tile([128, NCH * 128], BF16, tag=f"wkk{p}",
                              name=f"wkk{p}sb") for p in range(NPAIR)]
        wv_sb = consts.tile([128, NCH * 256], BF16, tag="wv", name="wvsb")
        wo_sb = consts.tile([128, 2 * 1024], F32R, tag="wo", name="wosb")
        xt_sb = [persist.tile([128, S], BF16, tag=f"xt{c}", name=f"xt{c}sb")
                 for c in range(NCH)]

        def dma_cols(sb, dram, lo, hi, pieces, bitcast=False, eng=None):
            eng = eng if eng is not None else nc.sync
            step_c = (hi - lo) // pieces
            for i in range(pieces):
                a, b_ = lo + i * step_c, lo + (i + 1) * step_c
                src = dram[:, a:b_]
                eng.dma_start(out=sb[:, a:b_],
                              in_=src.bitcast(F32R) if bitcast else src)

        # ---- input DMAs: weights on the SP DGE, x^T on the Activation DGE
        # (two hardware descriptor generators issue in parallel; ~0.6us per
        # dma_start on each sequencer is what paces the prologue) ----
        xtd = [d["xt"][c * 128:(c + 1) * 128, :] for c in range(NCH)]
        dma_cols(wqq_sb[0], d["wqq"][0], 0, 256, 2)
        dma_cols(xt_sb[0], xtd[0], 0, 512, 2, eng=nc.scalar)
        dma_cols(wqq_sb[0], d["wqq"][0], 256, 1024, 3)
        for c in range(1, NCH):
            dma_cols(xt_sb[c], xtd[c], 0, 512, 1, eng=nc.scalar)
        nc.sync.dma_start(out=bq_sb, in_=d["bq"][:])
        nc.sync.dma_start(out=bk_sb, in_=d["bk"][:])
        nc.sync.dma_start(out=nbias_sb, in_=d["nbias"][:])
        nc.sync.dma_start(out=mdiag_f32, in_=d["mdiag2"][:])
        nc.vector.tensor_copy(mdiag_sb, mdiag_f32)
        nc.sync.dma_start(out=swap_sb, in_=d["swap"][:].bitcast(F32R))
        dma_cols(wkk_sb[0], d["wkk"][0], 0, 1024, 4)
        dma_cols(wv_sb, d["wv"], 0, 2048, 4, eng=nc.scalar)
        dma_cols(wqq_sb[1], d["wqq"][1], 0, 1024, 4)
        dma_cols(wkk_sb[1], d["wkk"][1], 0, 1024, 4)
        for c in range(NCH):
            dma_cols(xt_sb[c], xtd[c], 512, 1024, 1)
        dma_cols(wo_sb, d["wo"], 0, 2048, 4, bitcast=True)
        for jb in (2, 3):
            for c in range(NCH):
                dma_cols(xt_sb[c], xtd[c], jb * 512, (jb + 1) * 512, 1)

        # ---- persistent activations ----
        qt_sb = []   # per pair: [128, S]; rows 0:64 head A Q^T, 64:128 head B
        kt_sb = []
        attnT = []   # per pair: [128, S]; normalized attn^T (dims on rows)
        for p in range(NPAIR):
            qt_sb.append(persist.tile([128, S], BF16, tag=f"qt{p}",
                                      name=f"qt{p}sb"))
            kt_sb.append(persist.tile([128, S], BF16, tag=f"kt{p}",
                                      name=f"kt{p}sb"))
            attnT.append(persist.tile([128, S], F32R, tag=f"at{p}",
                                      name=f"at{p}sb"))
        # V_aug [128, parity, pair, keycols]: even heads (A) = [V | ones],
        # odd heads (B) = [ones | V]
        vaug = persist.tile([128, 2, 2, NKC * 128], BF16, tag="vaug",
                            name="vaugsb")
        nc.gpsimd.memset(vaug, 1.0)

        # ---- PSUM pools: 4 (scores+proj) + 2 (outproj/den) + 2 (oA,oB) ----
        sp = top.enter_context(tc.tile_pool(name="sp", bufs=2, space="PSUM"))
        rot = top.enter_context(tc.tile_pool(name="rot", bufs=2, space="PSUM"))
        op = top.enter_context(tc.tile_pool(name="op", bufs=1, space="PSUM"))

        # ---- SBUF work pools ----
        ptp = top.enter_context(tc.tile_pool(name="ptp", bufs=6))
        nrm = top.enter_context(tc.tile_pool(name="nrm", bufs=3))
        ysb = top.enter_context(tc.tile_pool(name="ysb", bufs=4))

        def proj_units(jb, kinds="qkv"):
            """Q/K/V projection for query block jb: independent units."""
            jsl = slice(jb * 512, (jb + 1) * 512)
            units = []
            for p in range(NPAIR):
                for wsb, bsb, dst, nm in (
                    (wqq_sb[p], bq_sb, qt_sb[p], "q"),
                    (wkk_sb[p], bk_sb, kt_sb[p], "k"),
                ):
                    if nm not in kinds:
                        continue
                    def u(p=p, wsb=wsb, bsb=bsb, dst=dst, nm=nm, jsl=jsl, jb=jb):
                        ps = sp.tile([128, 2, 512], F32, tag="s",
                                     name=f"ps{nm}{p}{jb}")
                        for c in range(NCH):
                            nc.tensor.matmul(
                                ps[:, 0, :],
                                wsb[:, c * 128:(c + 1) * 128],
                                xt_sb[c][:, jsl],
                                start=(c == 0), stop=(c == NCH - 1),
                            )
                        nc.vector.tensor_scalar_add(dst[:, jsl], ps[:, 0, :],
                                                    bsb[:, p:p + 1])
                    units.append(u)
            if "v" not in kinds:
                return units
            for t in range(4 * jb, 4 * jb + 4):
                def u(t=t):
                    ps = sp.tile([128, 4, 128], F32, tag="s", name=f"psv{t}")
                    for c in range(NCH):
                        nc.tensor.matmul(
                            ps[:, 0:2, :],
                            xt_sb[c][:, t * 128:(t + 1) * 128],
                            wv_sb[:, c * 256:(c + 1) * 256],
                            start=(c == 0), stop=(c == NCH - 1),
                        )
                    # ps cols = [h0|h1|h2|h3] x 64; even heads' V to parity 0
                    # front half, odd heads' V to parity 1 back half.
                    nc.vector.tensor_copy(
                        vaug[:, 0, :, t * 128:t * 128 + 64], ps[:, 0:2, 0:64])
                    nc.vector.tensor_copy(
                        vaug[:, 1, :, t * 128 + 64:(t + 1) * 128],
                        ps[:, 0:2, 64:128])
                units.append(u)
            return units

        def psf_units(j, tail=False):
            """Output projection for query block j: 8 independent units."""
            units = []
            for q in range(4 * j, 4 * j + 4):
                for half in range(2):
                    def u(q=q, half=half, tail=tail):
                        pf = rot.tile([128, 512], F32, tag="r",
                                      name=f"pf{q}{half}")
                        for p in range(NPAIR):
                            nc.tensor.matmul(
                                pf,
                                _r(attnT[p][:, q * 128:(q + 1) * 128]),
                                _r(wo_sb[:, p * 1024 + half * 512:
                                         p * 1024 + half * 512 + 512]),
                                start=(p == 0), stop=(p == 1),
                            )
                        yt = ysb.tile([128, 512], BF16, tag="y",
                                      name=f"yt{q}{half}")
                        if tail:
                            nc.scalar.copy(yt, pf)
                            eng = nc.sync if (q + half) % 2 else nc.scalar
                            eng.dma_start(
                                out=d["y"][q * 128:(q + 1) * 128,
                                           half * 512:(half + 1) * 512],
                                in_=yt)
                        else:
                            nc.vector.tensor_copy(yt, pf)
                            for piece in range(2):
                                ysl = slice(piece * 256, (piece + 1) * 256)
                                nc.gpsimd.dma_start(
                                    out=d["y"][q * 128:(q + 1) * 128,
                                               half * 512 + piece * 256:
                                               half * 512 + (piece + 1) * 256],
                                    in_=yt[:, ysl])
                    units.append(u)
            return units

        def emit_scores(p, j, c):
            """Scores + exp (+ diag mask) for chunk c; returns attnV args."""
            t = c - 4 * j
            fo = 128 * t if t > 0 else 0
            w = 512 - fo
            qsl = slice(j * 512 + fo, (j + 1) * 512)
            with tc.high_priority(offset=150):
                s2 = sp.tile([128, 2, 512], F32, tag="s", name=f"s{p}{j}{c}")
                nc.tensor.matmul(s2[:, 0:1, :w],
                                 kt_sb[p][0:64, c * 128:(c + 1) * 128],
                                 qt_sb[p][0:64, qsl], start=True, stop=True)
                nc.tensor.matmul(s2[:, 1:2, :w],
                                 kt_sb[p][64:128, c * 128:(c + 1) * 128],
                                 qt_sb[p][64:128, qsl], start=True, stop=True)
                p2 = ptp.tile([128, 2, 512], BF16, tag="p", name=f"p{p}{j}{c}")
                nc.scalar.activation(p2[:, :, :w], s2[:, :, :w], EXP,
                                     bias=nbias_sb[:, c:c + 1], scale=SCALE)
            if t >= 0:
                # diagonal 128x128 block: zero keys below the diagonal for
                # both heads in one op
                nc.vector.tensor_mul(p2[:, :, 0:128], p2[:, :, 0:128],
                                     mdiag_sb)
            return p2, fo, w

        def emit_attnv(p, j, c, oA, oB, cmax, p2, fo, w):
            ksl = slice(c * 128, (c + 1) * 128)
            nc.tensor.matmul(oA[:, fo:512], vaug[:, 0, p, ksl],
                             p2[:, 0:1, :w], start=(c == 0), stop=(c == cmax))
            nc.tensor.matmul(oB[:, fo:512], vaug[:, 1, p, ksl],
                             p2[:, 1:2, :w], start=(c == 0), stop=(c == cmax))

        def emit_norm(p, j, oA, oB):
            # denA = oA[64:128], denB = oB[0:64]; swap halves on the PE so
            # each reciprocal lands on its numerator's partitions.
            jsl = slice(j * 512, (j + 1) * 512)
            scr = nrm.tile([128, 512], F32R, tag="scr", name=f"scr{p}{j}")
            nc.vector.tensor_copy(scr[64:128, :], oA[64:128, :])
            nc.vector.tensor_copy(scr[0:64, :], oB[0:64, :])
            den2 = rot.tile([128, 512], F32, tag="r", name=f"den{p}{j}")
            nc.tensor.matmul(den2, swap_sb, scr, start=True, stop=True)
            rec = nrm.tile([128, 512], F32, tag="rec", name=f"rec{p}{j}")
            nc.vector.reciprocal_approx_fast(out=rec, in_=den2)
            nc.vector.tensor_mul(attnT[p][0:64, jsl], oA[0:64, :],
                                 rec[0:64, :])
            nc.vector.tensor_mul(attnT[p][64:128, jsl], oB[64:128, :],
                                 rec[64:128, :])

        # ---- main schedule ----
        for u in proj_units(0):
            u()
        # Filler assignment keeps every step PE-bound.  Block 3's K and V
        # projections are only consumed from chunk 12 of step 3, so they
        # slide into step 3 itself as guaranteed-ready PE filler for its
        # exp-heavy stretch; out-projections lag two steps for the same
        # reason.
        for j in range(NJ):
            if j == 0:
                fillers = proj_units(1)
            elif j == 1:
                fillers = proj_units(2) + psf_units(0)
            elif j == 2:
                fillers = proj_units(3, kinds="q") + psf_units(1)
            else:
                fillers = proj_units(3, kinds="kv") + psf_units(2)
            nch_j = 4 * j + 4
            total_chunks = 2 * nch_j
            done = 0
            ci = 0
            for p in range(NPAIR):
                oA = op.tile([128, 512], F32, tag="oA", name=f"oA{p}{j}")
                oB = op.tile([128, 512], F32, tag="oB", name=f"oB{p}{j}")
                pend = []
                for c in range(nch_j):
                    pend.append((c,) + emit_scores(p, j, c))
                    if len(pend) > 2:
                        c0, p2, fo, w = pend.pop(0)
                        emit_attnv(p, j, c0, oA, oB, nch_j - 1, p2, fo, w)
                    ci += 1
                    want = ci * len(fillers) // total_chunks
                    while done < want:
                        fillers[done]()
                        done += 1
                for c0, p2, fo, w in pend:
                    emit_attnv(p, j, c0, oA, oB, nch_j - 1, p2, fo, w)
                emit_norm(p, j, oA, oB)
            while done < len(fillers):
                fillers[done]()
                done += 1
        for u in psf_units(NJ - 1, tail=True):
            u()

        if _DEBUG:
            for p in range(NPAIR):
                nc.sync.dma_start(out=d[f"dbg_qt{p}"][:], in_=qt_sb[p].bitcast(F32))
                nc.sync.dma_start(out=d[f"dbg_kt{p}"][:], in_=kt_sb[p].bitcast(F32))
                nc.sync.dma_start(out=d[f"dbg_at{p}"][:], in_=attnT[p].bitcast(F32))
            for h in range(4):
                nc.sync.dma_start(out=d[f"dbg_va{h}"][:], in_=vaug[h].bitcast(F32))


_NC_CACHE = {}
_DEBUG = False


def _get_nc():
    if "nc" not in _NC_CACHE:
        nc = bacc.Bacc(None, target_bir_lowering=False)
        d = {
            "xt": nc.dram_tensor("xt", [H, S], BF16, kind="ExternalInput"),
            "wqq": nc.dram_tensor("wqq", [NPAIR, 128, NCH * 128], BF16,
                                  kind="ExternalInput"),
            "wkk": nc.dram_tensor("wkk", [NPAIR, 128, NCH * 128], BF16,
                                  kind="ExternalInput"),
            "wv": nc.dram_tensor("wv", [128, NCH * 256], BF16, kind="ExternalInput"),
            "wo": nc.dram_tensor("wo", [128, 2 * 1024], F32, kind="ExternalInput"),
            "bq": nc.dram_tensor("bq", [128, 2], F32, kind="ExternalInput"),
            "bk": nc.dram_tensor("bk", [128, 2], F32, kind="ExternalInput"),
            "nbias": nc.dram_tensor("nbias", [128, NKC], F32, kind="ExternalInput"),
            "mdiag2": nc.dram_tensor("mdiag2", [128, 2, 128], F32,
                                     kind="ExternalInput"),
            "swap": nc.dram_tensor("swap", [128, 128], F32, kind="ExternalInput"),
            "y": nc.dram_tensor("y", [S, H], BF16, kind="ExternalOutput"),
        }
        if _DEBUG:
            for p in range(NPAIR):
                d[f"dbg_qt{p}"] = nc.dram_tensor(f"dbg_qt{p}", [128, S], F32,
                                                 kind="ExternalOutput")
                d[f"dbg_kt{p}"] = nc.dram_tensor(f"dbg_kt{p}", [128, S], F32,
                                                 kind="ExternalOutput")
                d[f"dbg_at{p}"] = nc.dram_tensor(f"dbg_at{p}", [128, S], F32,
                                                 kind="ExternalOutput")
            for h in range(4):
                d[f"dbg_va{h}"] = nc.dram_tensor(f"dbg_va{h}", [128, NKC * 128],
                                                 F32, kind="ExternalOutput")
        _emit(nc, d)
        nc.finalize()
        _NC_CACHE["nc"] = nc
    return _NC_CACHE["nc"]


def _chunked(w, ncols):
    """[H, ncols] -> [128, NCH*ncols] with chunk c of rows at cols c*ncols."""
    return np.ascontiguousarray(
        w.reshape(NCH, 128, ncols).transpose(1, 0, 2).reshape(128, NCH * ncols))


def _make_in_maps(batch, input_ids, W_Q, W_K, W_V, W_O, b_Q, b_K):
    m = np.triu(np.ones((128, 128), np.float32))
    mdiag2 = np.ascontiguousarray(np.stack([m, m], axis=1))  # [128, 2, 128]
    swap = np.zeros((128, 128), np.float32)
    swap[64:128, 0:64] = np.eye(64, dtype=np.float32)
    swap[0:64, 64:128] = np.eye(64, dtype=np.float32)
    in_maps = []
    for core in range(NCORE):
        b, g = divmod(core, 4)
        base = 256 * g  # first feature column of this core's 4 heads
        wqq = np.stack([_chunked(W_Q[:, base + 128 * p: base + 128 * (p + 1)], 128)
                        for p in range(NPAIR)])
        wkk = np.stack([_chunked(W_K[:, base + 128 * p: base + 128 * (p + 1)], 128)
                        for p in range(NPAIR)])
        wv = _chunked(W_V[:, base: base + 256], 256)
        wo = np.ascontiguousarray(
            W_O[base: base + 256, :].reshape(2, 128, H)
            .transpose(1, 0, 2).reshape(128, 2 * H))
        bq = np.stack([b_Q[base + 128 * p: base + 128 * (p + 1)]
                       for p in range(NPAIR)], axis=1)
        bk = np.stack([b_K[base + 128 * p: base + 128 * (p + 1)]
                       for p in range(NPAIR)], axis=1)
        keep = input_ids[b] != 0
        nbias = np.where(keep, 0.0, NEG_BIAS).astype(np.float32)
        nbias = np.ascontiguousarray(nbias.reshape(NKC, 128).T)
        xt = np.ascontiguousarray(batch[b].T)
        bf = ml_dtypes.bfloat16
        in_maps.append({
            "xt": xt.astype(bf), "wqq": wqq.astype(bf),
            "wkk": wkk.astype(bf), "wv": wv.astype(bf), "wo": wo,
            "bq": np.ascontiguousarray(bq), "bk": np.ascontiguousarray(bk),
            "nbias": nbias, "mdiag2": mdiag2, "swap": swap,
        })
    return in_maps


def _run(in_maps, **kwargs):
    nc = _get_nc()
    return run_bass_kernel_spmd(nc, in_maps, core_ids=list(range(NCORE)), **kwargs)


def kernel(batch, input_ids, W_Q, W_K, W_V, b_Q, b_K, b_V, W_O, b_O,
           _results_out=None, **run_kwargs):
    batch = np.asarray(batch, np.float32)
    input_ids = np.asarray(input_ids)
    W_Q, W_K, W_V = (np.asarray(a, np.float32) for a in (W_Q, W_K, W_V))
    b_Q, b_K, b_V = (np.asarray(a, np.float32) for a in (b_Q, b_K, b_V))
    W_O = np.asarray(W_O, np.float32)
    b_O = np.asarray(b_O, np.float32)

    in_maps = _make_in_maps(batch, input_ids, W_Q, W_K, W_V, W_O, b_Q, b_K)
    res = _run(in_maps, **run_kwargs)
    if _results_out is not None:
        _results_out.append(res)
    ys = [np.asarray(res.results[c]["y"], np.float32) for c in range(NCORE)]
    out = np.stack([sum(ys[4 * b: 4 * b + 4]) for b in range(B)], axis=0)
    # b_V enters as attn@1 * b_V = b_V (softmax rows sum to 1), then @ W_O.
    const_row = (b_V @ W_O + b_O).astype(np.float32)
    return (out + const_row).astype(np.float32)
